# revision 12
# baseline (speedup 1.0000x reference)
"""Trainium2 Bass kernel for nn_Attention_75814762709205.

Computation (per batch row b, seq s):
    proj  = hidden_enc @ W + b          [B,S,D]
    score = hidden_dec.T * proj         (per-channel scale)
    attn  = softmax(score, axis=-1)     (over D)
    out   = sum_s attn * hidden_enc     [B,D]

Sharding: data-parallel over batch, 4 batches per core on 8 cores.
W is pre-scaled by dec on device (score = enc @ (W*dec) [+ b*dec]), so
softmax input comes straight out of the matmul. The softmax denominator
is folded into the final sequence reduction: context = sum_s r_s *
(exp_s * enc_s) with r = 1/sum(exp), computed as a matmul with r as the
stationary vector (reduces over the 128 partition rows).
"""

import sys

sys.path.insert(0, "/opt/trn_rl_repo")

import numpy as np

import concourse.bass as bass
import concourse.mybir as mybir
import concourse.tile as tile
from concourse import bacc, bass_utils
from concourse.masks import make_identity

B, S, D = 32, 2048, 1024
NCORES = 8
BPC = B // NCORES  # batches per core
ROWS = BPC * S  # rows per core
P = 128
NT = ROWS // P  # row tiles per core
TPB = S // P  # row tiles per batch
KC = D // P  # contraction chunks
NHALF = 512  # matmul free-dim (one PSUM bank of fp32)
NH = D // NHALF

F32 = mybir.dt.float32
F32R = mybir.dt.float32r
BF16 = mybir.dt.bfloat16
AF = mybir.ActivationFunctionType


def r32(ap):
    return ap.bitcast(F32R)


def build_program(with_bias: bool, repeats: int = 1):
    nc = bacc.Bacc("TRN2", target_bir_lowering=False, debug=False)
    enc_d = nc.dram_tensor("enc", [ROWS, D], F32, kind="ExternalInput")
    w_d = nc.dram_tensor("w", [D, D], F32, kind="ExternalInput")
    dec_d = nc.dram_tensor("dec", [D, 1], F32, kind="ExternalInput")
    b_d = None
    if with_bias:
        b_d = nc.dram_tensor("b", [1, D], F32, kind="ExternalInput")
    out_d = nc.dram_tensor("out", [BPC, D], F32, kind="ExternalOutput")

    with tile.TileContext(nc) as tc:
        with (
            tc.tile_pool(name="consts", bufs=1) as consts,
            tc.tile_pool(name="wpool", bufs=1) as wpool,
            tc.tile_pool(name="encp", bufs=6) as encp,
            tc.tile_pool(name="encbp", bufs=6) as encbp,
            tc.tile_pool(name="encTp", bufs=6) as encTp,
            tc.tile_pool(name="expp", bufs=4) as expp,
            tc.tile_pool(name="wtp", bufs=4) as wtp,
            tc.tile_pool(name="smalls", bufs=12) as smalls,
            tc.tile_pool(name="outp", bufs=2) as outp,
            tc.tile_pool(name="pr_ps", bufs=3, space=bass.MemorySpace.PSUM) as pr_ps,
            tc.tile_pool(name="ctx_ps", bufs=2, space=bass.MemorySpace.PSUM) as ctx_ps,
        ):
            # dec broadcast to all partitions
            dec_row = consts.tile([1, D], F32)
            nc.sync.dma_start(dec_row, dec_d.ap().rearrange("d one -> one d"))
            dec_b = consts.tile([P, D], F32)
            nc.gpsimd.partition_broadcast(dec_b, dec_row)

            # W_eff[d, e] = W[d, e] * dec[e], stored as 8 chunks of [128, D]
            w_sb = wpool.tile([P, KC, D], BF16)
            for k in range(KC):
                w_stage = consts.tile([P, D], F32, name=f"wstage{k}")
                nc.sync.dma_start(w_stage, w_d.ap()[k * P : (k + 1) * P, :])
                nc.vector.tensor_mul(w_sb[:, k], w_stage, dec_b)

            ones_row = None
            b_eff = None
            if with_bias:
                ones_f32 = consts.tile([1, P], F32)
                nc.any.memset(ones_f32, 1.0)
                ones_row = consts.tile([1, P], F32R)
                nc.vector.tensor_copy(ones_row, ones_f32)
                b_row = consts.tile([1, D], F32)
                nc.sync.dma_start(b_row, b_d.ap())
                b_eff = consts.tile([1, D], F32R)
                nc.vector.tensor_mul(b_eff, b_row, dec_row)

            # Software pipeline: tile t's context matmuls (which depend on
            # exp -> recip -> wt of tile t) are emitted between tile t+1's
            # proj matmul halves so the PE never stalls on that tail.
            state = {"ctx_half": None}

            def emit_ctx(prev):
                recip, wt, tib, bidx = prev
                if tib == 0:
                    state["ctx_half"] = [
                        ctx_ps.tile([1, NHALF], F32, name=f"ctxh{h}")
                        for h in range(NH)
                    ]
                for h2 in range(NH):
                    nc.tensor.matmul(
                        state["ctx_half"][h2],
                        recip,
                        wt[:, h2 * NHALF : (h2 + 1) * NHALF],
                        start=(tib == 0),
                        stop=(tib == TPB - 1),
                    )
                if tib == TPB - 1:
                    ctx_sb = outp.tile([1, D], F32, name="ctx_sb")
                    for h2 in range(NH):
                        nc.vector.tensor_copy(
                            ctx_sb[:, h2 * NHALF : (h2 + 1) * NHALF],
                            state["ctx_half"][h2],
                        )
                    nc.sync.dma_start(out_d.ap()[bidx : bidx + 1, :], ctx_sb)

            from collections import deque

            # Prepare stage (load -> bf16 cast -> xbar transpose) runs LOOK
            # tiles ahead of the consume stage so the transpose chain never
            # sits on the exp/proj critical cycle.
            LOOK = 3
            prepared = {}

            def prepare(t):
                enc_t = encp.tile([P, D], F32, name="enc_t")
                nc.gpsimd.dma_start(enc_t, enc_d.ap()[(t % NT) * P : (t % NT + 1) * P, :])
                enc_bf = encbp.tile([P, D], BF16, name="enc_bf")
                nc.vector.tensor_copy(enc_bf, enc_t)
                encT = encTp.tile([P, KC, P], BF16, name="encT")
                nc.sync.dma_start_transpose(encT, enc_bf)
                prepared[t] = (enc_t, encT)

            pending = deque()
            for rep in range(repeats):
              for tt in range(NT):
                t = rep * NT + tt
                bidx, tib = divmod(tt, TPB)
                if t == 0:
                    for i in range(LOOK):
                        prepare(i)
                if t + LOOK < repeats * NT:
                    prepare(t + LOOK)
                enc_t, encT = prepared.pop(t)

                # proj = enc @ W_eff (+ b_eff), then exp with fused row-sum
                sums = smalls.tile([P, 2], F32)
                exp_t = expp.tile([P, D], F32)
                wt = wtp.tile([P, D], F32R)
                for h2 in range(NH):
                    pr = pr_ps.tile([P, NHALF], F32)
                    ncol = slice(h2 * NHALF, (h2 + 1) * NHALF)
                    for k in range(KC):
                        nc.tensor.matmul(
                            pr,
                            encT[:, k, :],
                            w_sb[:, k, ncol],
                            start=(k == 0),
                            stop=(k == KC - 1 and not with_bias),
                        )
                    if with_bias:
                        nc.tensor.matmul(
                            pr,
                            ones_row,
                            b_eff[:, ncol],
                            start=False,
                            stop=True,
                        )
                    if h2 == 0 and len(pending) >= 2:
                        emit_ctx(pending.popleft())
                    nc.scalar.activation(
                        exp_t[:, ncol], pr, AF.Exp, accum_out=sums[:, h2 : h2 + 1]
                    )
                    nc.vector.tensor_mul(wt[:, ncol], exp_t[:, ncol], enc_t[:, ncol])

                ssum = smalls.tile([P, 1], F32)
                nc.vector.tensor_add(ssum, sums[:, 0:1], sums[:, 1:2])
                recip_f = smalls.tile([P, 1], F32)
                nc.vector.reciprocal(recip_f, ssum)
                recip = smalls.tile([P, 1], F32R)
                nc.vector.tensor_copy(recip, recip_f)

                pending.append((recip, wt, tib, bidx))
              while pending:
                emit_ctx(pending.popleft())

    nc.compile()
    return nc


def make_in_maps(hidden_dec, hidden_enc, W, b):
    enc = np.asarray(hidden_enc, dtype=np.float32).reshape(B, S, D)
    W = np.asarray(W, dtype=np.float32)
    dec = np.asarray(hidden_dec, dtype=np.float32).reshape(D, 1)
    b = np.asarray(b, dtype=np.float32).reshape(D)
    with_bias = bool(np.any(b != 0.0))
    in_maps = []
    for c in range(NCORES):
        m = {
            "enc": np.ascontiguousarray(
                enc[c * BPC : (c + 1) * BPC].reshape(ROWS, D)
            ),
            "w": W,
            "dec": dec,
        }
        if with_bias:
            m["b"] = b.reshape(1, D)
        in_maps.append(m)
    return in_maps, with_bias


def kernel(hidden_dec, hidden_enc, W, b):
    in_maps, with_bias = make_in_maps(hidden_dec, hidden_enc, W, b)
    nc = build_program(with_bias)
    res = bass_utils.run_bass_kernel_spmd(nc, in_maps, core_ids=list(range(NCORES)))
    out = np.concatenate([res.results[c]["out"] for c in range(NCORES)], axis=0)
    return out.astype(np.float32)


# revision 13
# speedup vs baseline: 1.0538x; 1.0538x over previous
"""Trainium2 Bass kernel for nn_Attention_75814762709205.

Computation (per batch row b, seq s):
    proj  = hidden_enc @ W + b          [B,S,D]
    score = hidden_dec.T * proj         (per-channel scale)
    attn  = softmax(score, axis=-1)     (over D)
    out   = sum_s attn * hidden_enc     [B,D]

Sharding: data-parallel over batch, 4 batches per core on 8 cores.
W is pre-scaled by dec on device (score = enc @ (W*dec) [+ b*dec]), so
softmax input comes straight out of the matmul (fp32r, full speed on
PE).  The softmax denominator is folded into the final sequence
reduction: context = sum_s r_s * (exp_s * enc_s) with r = 1/sum(exp),
computed as a matmul with r as the stationary vector (reduces over the
128 partition rows).

Pipeline structure: enc tiles are transposed on the PE (via identity)
one tile ahead of their use, and the ctx matmuls run two tiles behind,
so the PE never waits on the ACT/DVE softmax tail.
"""

import sys

sys.path.insert(0, "/opt/trn_rl_repo")

from collections import deque

import numpy as np

import concourse.bass as bass
import concourse.mybir as mybir
import concourse.tile as tile
from concourse import bacc, bass_utils
from concourse.masks import make_identity

B, S, D = 32, 2048, 1024
NCORES = 8
BPC = B // NCORES  # batches per core
ROWS = BPC * S  # rows per core
P = 128
NT = ROWS // P  # row tiles per core
TPB = S // P  # row tiles per batch
KC = D // P  # contraction chunks
NHALF = 512  # matmul free-dim (one PSUM bank of fp32)
NH = D // NHALF

F32 = mybir.dt.float32
F32R = mybir.dt.float32r
AF = mybir.ActivationFunctionType


def build_program(with_bias: bool, repeats: int = 1):
    nc = bacc.Bacc("TRN2", target_bir_lowering=False, debug=False)
    enc_d = nc.dram_tensor("enc", [ROWS, D], F32, kind="ExternalInput")
    w_d = nc.dram_tensor("w", [D, D], F32, kind="ExternalInput")
    dec_d = nc.dram_tensor("dec", [D, 1], F32, kind="ExternalInput")
    b_d = None
    if with_bias:
        b_d = nc.dram_tensor("b", [1, D], F32, kind="ExternalInput")
    out_d = nc.dram_tensor("out", [BPC, D], F32, kind="ExternalOutput")

    NTOT = repeats * NT

    with tile.TileContext(nc) as tc:
        with (
            tc.tile_pool(name="consts", bufs=1) as consts,
            tc.tile_pool(name="wpool", bufs=1) as wpool,
            tc.tile_pool(name="encp", bufs=5) as encp,
            tc.tile_pool(name="encTp", bufs=3) as encTp,
            tc.tile_pool(name="expp", bufs=3) as expp,
            tc.tile_pool(name="wtp", bufs=4) as wtp,
            tc.tile_pool(name="smalls", bufs=12) as smalls,
            tc.tile_pool(name="outp", bufs=2) as outp,
            tc.tile_pool(name="tp_ps", bufs=2, space=bass.MemorySpace.PSUM) as tp_ps,
            tc.tile_pool(name="pr_ps", bufs=2, space=bass.MemorySpace.PSUM) as pr_ps,
            tc.tile_pool(name="ctx_ps", bufs=2, space=bass.MemorySpace.PSUM) as ctx_ps,
        ):
            identity = consts.tile([P, P], F32)
            make_identity(nc, identity)

            # dec broadcast to all partitions
            dec_row = consts.tile([1, D], F32)
            nc.sync.dma_start(dec_row, dec_d.ap().rearrange("d one -> one d"))
            dec_b = consts.tile([P, D], F32)
            nc.gpsimd.partition_broadcast(dec_b, dec_row)

            # W_eff[d, e] = W[d, e] * dec[e], stored as 8 chunks of [128, D]
            w_sb = wpool.tile([P, KC, D], F32R)
            for k in range(KC):
                w_stage = consts.tile([P, D], F32, name=f"wstage{k}")
                nc.sync.dma_start(w_stage, w_d.ap()[k * P : (k + 1) * P, :])
                nc.vector.tensor_mul(w_sb[:, k], w_stage, dec_b)

            ones_row = None
            b_eff = None
            if with_bias:
                ones_f32 = consts.tile([1, P], F32)
                nc.any.memset(ones_f32, 1.0)
                ones_row = consts.tile([1, P], F32R)
                nc.vector.tensor_copy(ones_row, ones_f32)
                b_row = consts.tile([1, D], F32)
                nc.sync.dma_start(b_row, b_d.ap())
                b_eff = consts.tile([1, D], F32R)
                nc.vector.tensor_mul(b_eff, b_row, dec_row)

            # ---- pipeline stages ----
            loaded = {}

            def load(t):
                enc_t = encp.tile([P, D], F32, name="enc_t")
                nc.sync.dma_start(enc_t, enc_d.ap()[(t % NT) * P : (t % NT + 1) * P, :])
                loaded[t] = enc_t

            prepared = {}

            def transpose(t):
                enc_t = loaded[t]
                encT = encTp.tile([P, D], F32R, name="encT")
                for h in range(2):
                    tp = tp_ps.tile([P, NHALF], F32, name="tp")
                    for j in range(4):
                        c = h * 4 + j
                        nc.tensor.transpose(
                            tp[:, j * P : (j + 1) * P],
                            enc_t[:, c * P : (c + 1) * P],
                            identity,
                        )
                    nc.scalar.copy(encT[:, h * NHALF : (h + 1) * NHALF], tp)
                prepared[t] = encT

            state = {"ctx_half": None}

            def emit_ctx(prev):
                recip, wt, tib, bidx = prev
                if tib == 0:
                    state["ctx_half"] = [
                        ctx_ps.tile([1, NHALF], F32, name=f"ctxh{h}")
                        for h in range(NH)
                    ]
                for h2 in range(NH):
                    nc.tensor.matmul(
                        state["ctx_half"][h2],
                        recip,
                        wt[:, h2 * NHALF : (h2 + 1) * NHALF],
                        start=(tib == 0),
                        stop=(tib == TPB - 1),
                    )
                if tib == TPB - 1:
                    ctx_sb = outp.tile([1, D], F32, name="ctx_sb")
                    for h2 in range(NH):
                        nc.vector.tensor_copy(
                            ctx_sb[:, h2 * NHALF : (h2 + 1) * NHALF],
                            state["ctx_half"][h2],
                        )
                    nc.sync.dma_start(out_d.ap()[bidx : bidx + 1, :], ctx_sb)

            pending = deque()
            for t in range(NTOT):
                bidx, tib = divmod(t % NT, TPB)
                if t == 0:
                    load(0)
                    load(1)
                    transpose(0)
                if t + 2 < NTOT:
                    load(t + 2)
                if t + 1 < NTOT:
                    transpose(t + 1)  # PE: transposes for t+1 ahead of proj t
                enc_t = loaded.pop(t)
                encT = prepared.pop(t)

                # proj = enc @ W_eff (+ b_eff), then exp with fused row-sum
                sums = smalls.tile([P, 2], F32)
                exp_t = expp.tile([P, D], F32)
                wt = wtp.tile([P, D], F32R)
                for h2 in range(NH):
                    pr = pr_ps.tile([P, NHALF], F32)
                    ncol = slice(h2 * NHALF, (h2 + 1) * NHALF)
                    for k in range(KC):
                        nc.tensor.matmul(
                            pr,
                            encT[:, k * P : (k + 1) * P],
                            w_sb[:, k, ncol],
                            start=(k == 0),
                            stop=(k == KC - 1 and not with_bias),
                        )
                    if with_bias:
                        nc.tensor.matmul(
                            pr, ones_row, b_eff[:, ncol], start=False, stop=True
                        )
                    if h2 == 0 and len(pending) >= 2:
                        emit_ctx(pending.popleft())
                    nc.scalar.activation(
                        exp_t[:, ncol], pr, AF.Exp, accum_out=sums[:, h2 : h2 + 1]
                    )
                    nc.vector.tensor_mul(wt[:, ncol], exp_t[:, ncol], enc_t[:, ncol])

                ssum = smalls.tile([P, 1], F32)
                nc.vector.tensor_add(ssum, sums[:, 0:1], sums[:, 1:2])
                recip_f = smalls.tile([P, 1], F32)
                nc.vector.reciprocal(recip_f, ssum)
                recip = smalls.tile([P, 1], F32R)
                nc.vector.tensor_copy(recip, recip_f)

                pending.append((recip, wt, tib, bidx))
            while pending:
                emit_ctx(pending.popleft())

    nc.compile()
    return nc


def make_in_maps(hidden_dec, hidden_enc, W, b):
    enc = np.asarray(hidden_enc, dtype=np.float32).reshape(B, S, D)
    W = np.asarray(W, dtype=np.float32)
    dec = np.asarray(hidden_dec, dtype=np.float32).reshape(D, 1)
    b = np.asarray(b, dtype=np.float32).reshape(D)
    with_bias = bool(np.any(b != 0.0))
    in_maps = []
    for c in range(NCORES):
        m = {
            "enc": np.ascontiguousarray(
                enc[c * BPC : (c + 1) * BPC].reshape(ROWS, D)
            ),
            "w": W,
            "dec": dec,
        }
        if with_bias:
            m["b"] = b.reshape(1, D)
        in_maps.append(m)
    return in_maps, with_bias


def kernel(hidden_dec, hidden_enc, W, b):
    in_maps, with_bias = make_in_maps(hidden_dec, hidden_enc, W, b)
    nc = build_program(with_bias)
    res = bass_utils.run_bass_kernel_spmd(nc, in_maps, core_ids=list(range(NCORES)))
    out = np.concatenate([res.results[c]["out"] for c in range(NCORES)], axis=0)
    return out.astype(np.float32)
